# revision 36
# baseline (speedup 1.0000x reference)
"""ExpertGroupRouter MoE routing kernel for 8 TRN2 NeuronCores (Bass/Tile).

Strategy (data-parallel over tokens):
  - Flatten x to (16384, 2048) tokens; shard 2048 contiguous tokens per core.
  - Per core: stream x in 16 tiles of (128 tok, 2048 d). PE-transpose each
    128x128 block to build xT (d-major) chunks, copy PSUM->SBUF (DVE/ACT
    alternating), then PE matmul with the replicated 18-col weight matrix
    W^T (W_expert ++ W_group) to get scoresT (18, tok) with f32 PSUM
    accumulation over the 16 d-chunks.  float32r (11-bit-mantissa fp32) is
    used on the PE for 4x matmul and ~2.5x transpose throughput; the
    routing decisions stay f32.
  - scoresT is PE-transposed back to token-major (128, 18) tiles and the
    full routing logic (grouped softmax top-1/top-1/top-2, sigmoid gates,
    weight normalization, expert bincount) runs batched on DVE/ACT, split
    into two halves so it overlaps the score streaming.
  - x loads alternate between the SP and ACT hardware DGE rings to use
    both HWDGE queues.
  - Host gathers the 8 shards, sums the per-core/per-partition expert
    counts and computes the scalar KL aux loss from the 16 totals.
"""
import sys

if "/opt/trn_rl_repo" not in sys.path:
    sys.path.insert(0, "/opt/trn_rl_repo")

import numpy as np

B, T, D, E = 4, 4096, 2048, 16
N_CORES = 8
TOK = B * T                  # 16384 tokens
TPC = TOK // N_CORES         # 2048 tokens per core
N_TILES = TPC // 128         # 16 token tiles per core
N_CHUNKS = D // 128          # 16 d-chunks
GROUPS = N_TILES // 4        # 4 groups of 512 tokens
BIG = 65536.0
THR = 0.15
N_HALVES = 2                 # routing passes (overlap with streaming)

_cache = {}


def _round_f32r(a: np.ndarray) -> np.ndarray:
    """Round f32 bits to the PE's fp32r format (RNE to 11 mantissa bits)."""
    u = a.astype(np.float32).view(np.uint32).astype(np.uint64)
    u = (u + 0x7FF + ((u >> 12) & 1)) & 0xFFFFF000
    return (u & 0xFFFFFFFF).astype(np.uint32).view(np.float32)


def _build():
    if "nc" in _cache:
        return _cache["nc"]
    import concourse.bass as bass
    import concourse.tile as tile
    from concourse import mybir, bacc

    F32, F32R, I32 = mybir.dt.float32, mybir.dt.float32r, mybir.dt.int32
    AX = mybir.AxisListType
    OP = mybir.AluOpType
    ACTF = mybir.ActivationFunctionType

    nc = bacc.Bacc("TRN2", target_bir_lowering=False, debug=False)
    x_d = nc.dram_tensor("x", [TPC, D], F32R, kind="ExternalInput")
    # wT packed host-side into the SBUF layout: (128, N_CHUNKS*18)
    wT_d = nc.dram_tensor("wT", [128, N_CHUNKS * 18], F32R, kind="ExternalInput")
    id_d = nc.dram_tensor("ident", [128, 128], F32R, kind="ExternalInput")
    idb_d = nc.dram_tensor("ident_bf", [128, 128], mybir.dt.bfloat16,
                           kind="ExternalInput")
    iota_d = nc.dram_tensor("iota", [128, 16 * 16], F32, kind="ExternalInput")
    ow_d = nc.dram_tensor("out_w", [128, N_TILES * 6], F32, kind="ExternalOutput")
    oi_d = nc.dram_tensor("out_i", [128, N_TILES * 6], I32, kind="ExternalOutput")
    oc_d = nc.dram_tensor("out_c", [128, N_HALVES * E], F32, kind="ExternalOutput")

    import contextlib

    with tile.TileContext(nc) as tc, contextlib.ExitStack() as ctx:
        const = ctx.enter_context(tc.tile_pool(name="const", bufs=1))
        xpool = ctx.enter_context(tc.tile_pool(name="xpool", bufs=12))
        xtp = ctx.enter_context(tc.tile_pool(name="xtp", bufs=4))
        pxT = ctx.enter_context(tc.tile_pool(name="pxT", bufs=3, space="PSUM"))
        psc = ctx.enter_context(tc.tile_pool(name="psc", bufs=2, space="PSUM"))
        pbt = ctx.enter_context(tc.tile_pool(name="pbt", bufs=2, space="PSUM"))
        sc_sb = ctx.enter_context(tc.tile_pool(name="sc_sb", bufs=2))
        rt = ctx.enter_context(tc.tile_pool(name="rt", bufs=1))
        scr = ctx.enter_context(tc.tile_pool(name="scr", bufs=2))

        # ---- constants (identity first — transposes need it; the rest
        # is loaded on the gpsimd ring so the sync ring starts on x) ----
        idt = const.tile([128, 128], F32R)
        nc.sync.dma_start(idt[:], id_d.ap()[:])
        wTt = const.tile([128, N_CHUNKS * 18], F32R)
        nc.gpsimd.dma_start(wTt[:], wT_d.ap()[:])
        iot = const.tile([128, 16 * 16], F32)
        nc.gpsimd.dma_start(iot[:], iota_d.ap()[:])

        # persistent tiles
        S_all = rt.tile([128, N_TILES * 18], F32)   # token-major scores
        ow_sb = rt.tile([128, N_TILES * 6], F32)
        oi_sb = rt.tile([128, N_TILES * 6], I32)
        cnt = rt.tile([128, N_HALVES * E], F32)

        io3 = iot[:].rearrange("p (t e) -> p t e", t=16)

        def slot(view3, j):
            return view3[:, :, j:j + 1].rearrange("p t a -> p (t a)")

        # ---------------- routing block over tiles [ts, ts+tn) ----------
        def routing_block(h, ts, tn):
            S = S_all[:, ts * 18:(ts + tn) * 18].rearrange(
                "p (t e) -> p t e", t=tn
            )
            ioA = io3[:, 0:tn, 0:8]
            ioB = io3[:, 0:tn, 8:12]
            ioC = io3[:, 0:tn, 12:16]

            def bcast(t2d, w):
                return t2d[:].to_broadcast([128, tn, w])

            sa, sb4, sc4 = S[:, :, 0:8], S[:, :, 8:12], S[:, :, 12:16]
            gsc = S[:, :, 16:18]

            def tl(name, wdt=1, dt=F32):
                return scr.tile([128, tn * wdt], dt, tag=name, name=f"{name}_h{h}")

            m_a, m_b, m_c = tl("m_a"), tl("m_b"), tl("m_c")
            nc.vector.reduce_max(m_a[:], sa, axis=AX.X)
            nc.vector.reduce_max(m_b[:], sb4, axis=AX.X)
            nc.vector.reduce_max(m_c[:], sc4, axis=AX.X)

            E_all = tl("E_all", 16)
            E3 = E_all[:].rearrange("p (t e) -> p t e", t=tn)
            nc.scalar.activation(E3, S[:, :, 0:16], ACTF.Exp)
            sum_a, sum_b, sum_c = tl("sum_a"), tl("sum_b"), tl("sum_c")
            nc.vector.reduce_sum(sum_a[:], E3[:, :, 0:8], axis=AX.X)
            nc.vector.reduce_sum(sum_b[:], E3[:, :, 8:12], axis=AX.X)
            nc.vector.reduce_sum(sum_c[:], E3[:, :, 12:16], axis=AX.X)

            em_a, em_b, em_c = tl("em_a"), tl("em_b"), tl("em_c")
            nc.scalar.activation(em_a[:], m_a[:], ACTF.Exp)
            nc.scalar.activation(em_b[:], m_b[:], ACTF.Exp)
            nc.scalar.activation(em_c[:], m_c[:], ACTF.Exp)

            ra, rb, rc = tl("ra"), tl("rb"), tl("rc")
            nc.vector.reciprocal(ra[:], sum_a[:])
            nc.vector.reciprocal(rb[:], sum_b[:])
            nc.vector.reciprocal(rc[:], sum_c[:])

            gs = tl("gs", 2)
            g3 = gs[:].rearrange("p (t g) -> p t g", t=tn)
            nc.scalar.activation(g3, gsc, ACTF.Sigmoid)

            gm0, gm1, msk = tl("gm0"), tl("gm1"), tl("msk")
            g0 = g3[:, :, 0:1].rearrange("p t a -> p (t a)")
            g1 = g3[:, :, 1:2].rearrange("p t a -> p (t a)")
            nc.vector.tensor_scalar(msk[:], g0, THR, None, op0=OP.is_gt)
            nc.vector.tensor_tensor(gm0[:], msk[:], g0, op=OP.mult)
            nc.vector.tensor_scalar(msk[:], g1, THR, None, op0=OP.is_gt)
            nc.vector.tensor_tensor(gm1[:], msk[:], g1, op=OP.mult)

            scr8 = tl("scr8", 8)
            s8 = scr8[:].rearrange("p (t e) -> p t e", t=tn)
            scr4 = tl("scr4", 4)
            s4 = scr4[:].rearrange("p (t e) -> p t e", t=tn)

            def argmax_idx(out, s_view, m_t, io_view, w, sc3):
                nc.vector.tensor_tensor(sc3, s_view, bcast(m_t, w), op=OP.is_lt)
                nc.vector.scalar_tensor_tensor(
                    sc3, sc3, BIG, io_view, op0=OP.mult, op1=OP.add
                )
                nc.vector.tensor_reduce(out, sc3, axis=AX.X, op=OP.min)

            idx_a, idx_b = tl("idx_a"), tl("idx_b")
            idx_c1, idx_c2 = tl("idx_c1"), tl("idx_c2")
            argmax_idx(idx_a[:], sa, m_a, ioA, 8, s8)
            argmax_idx(idx_b[:], sb4, m_b, ioB, 4, s4)
            argmax_idx(idx_c1[:], sc4, m_c, ioC, 4, s4)

            sc_m = tl("sc_m", 4)
            sm4 = sc_m[:].rearrange("p (t e) -> p t e", t=tn)
            nc.vector.tensor_tensor(s4, ioC, bcast(idx_c1, 4), op=OP.is_equal)
            nc.vector.scalar_tensor_tensor(sm4, s4, -BIG, sc4, op0=OP.mult, op1=OP.add)
            m_c2 = tl("m_c2")
            nc.vector.reduce_max(m_c2[:], sm4, axis=AX.X)
            argmax_idx(idx_c2[:], sm4, m_c2, ioC, 4, s4)
            em_c2 = tl("em_c2")
            nc.scalar.activation(em_c2[:], m_c2[:], ACTF.Exp)

            # weights
            W_raw = tl("W_raw", 6)
            W3 = W_raw[:].rearrange("p (t s) -> p t s", t=tn)
            nc.vector.memset(W_raw[:], 0.0)
            tb = tl("tb")
            nc.vector.tensor_tensor(slot(W3, 0), em_a[:], ra[:], op=OP.mult)
            nc.vector.tensor_tensor(tb[:], em_b[:], rb[:], op=OP.mult)
            nc.vector.tensor_tensor(slot(W3, 1), tb[:], gm0[:], op=OP.mult)
            nc.vector.tensor_tensor(tb[:], em_c[:], rc[:], op=OP.mult)
            nc.vector.tensor_tensor(slot(W3, 2), tb[:], gm1[:], op=OP.mult)
            nc.vector.tensor_tensor(tb[:], em_c2[:], rc[:], op=OP.mult)
            nc.vector.tensor_tensor(slot(W3, 3), tb[:], gm1[:], op=OP.mult)

            sum_w, winv = tl("sum_w"), tl("winv")
            nc.vector.reduce_sum(sum_w[:], W3[:, :, 0:4], axis=AX.X)
            nc.vector.tensor_scalar(sum_w[:], sum_w[:], 1e-8, None, op0=OP.add)
            nc.vector.reciprocal(winv[:], sum_w[:])

            ow3 = ow_sb[:, ts * 6:(ts + tn) * 6].rearrange("p (t s) -> p t s", t=tn)
            nc.vector.tensor_tensor(ow3, W3, bcast(winv, 6), op=OP.mult)
            nc.sync.dma_start(
                ow_d.ap()[:, ts * 6:(ts + tn) * 6], ow_sb[:, ts * 6:(ts + tn) * 6]
            )

            # indices
            I_f = tl("I_f", 6)
            I3 = I_f[:].rearrange("p (t s) -> p t s", t=tn)
            nc.vector.memset(I_f[:], 0.0)
            nc.vector.tensor_copy(slot(I3, 0), idx_a[:])
            nc.vector.tensor_copy(slot(I3, 1), idx_b[:])
            nc.vector.tensor_copy(slot(I3, 2), idx_c1[:])
            nc.vector.tensor_copy(slot(I3, 3), idx_c2[:])
            oi_slice = oi_sb[:, ts * 6:(ts + tn) * 6]
            nc.vector.tensor_copy(oi_slice, I_f[:])
            nc.sync.dma_start(oi_d.ap()[:, ts * 6:(ts + tn) * 6], oi_slice)

            # counts
            cdummy = tl("cdummy", 4)
            cd3 = cdummy[:].rearrange("p (t s) -> p t s", t=tn)
            for e in range(E):
                nc.vector.tensor_scalar(
                    cd3, I3[:, :, 0:4], float(e), 0.0, op0=OP.is_equal, op1=OP.add,
                    accum_out=cnt[:, h * E + e:h * E + e + 1],
                )

        # ---------------- main streaming loop (software-pipelined) ------
        # Per global chunk slot j (group g = j//16, chunk k = j%16):
        #   - issue the group's x-tile DMAs at its first slot
        #   - emit 4 PE transposes for chunk j and the PSUM->SBUF copy
        #   - emit the score matmul for chunk j-MM_LAG (so the PE, which
        #     executes in order, never waits on an in-flight copy)
        MM_LAG = 2
        N_SLOTS = GROUPS * N_CHUNKS
        xts = {}         # group -> list of 4 x tiles
        pos = {}         # group -> psum scores tile
        sts = {}         # global chunk j -> xT sbuf tile

        def emit_group_epilogue(g):
            # scoresT (18, 512) -> token-major (128, 18) tiles
            sct = sc_sb.tile([18, 512], F32, tag="sct", name=f"sct_{g}")
            nc.scalar.copy(sct[:], pos[g][:])
            for tt in range(4):
                pb = pbt.tile([128, 18], F32, tag="pb", name=f"pb_{g}_{tt}")
                nc.tensor.transpose(
                    pb[:], sct[:, tt * 128:(tt + 1) * 128],
                    idt[0:18, 0:18].bitcast(F32),
                )
                dst = S_all[:, (g * 4 + tt) * 18:(g * 4 + tt + 1) * 18]
                nc.scalar.copy(dst, pb[:])
            if g == GROUPS // 2 - 1:
                routing_block(0, 0, N_TILES // 2)
            elif g == GROUPS - 1:
                routing_block(1, N_TILES // 2, N_TILES // 2)

        for j in range(N_SLOTS + MM_LAG):
            g, k = j // N_CHUNKS, j % N_CHUNKS
            if j < N_SLOTS:
                if k == 0:
                    xt = []
                    for tt in range(4):
                        t = xpool.tile([128, D], F32R, tag="xt",
                                       name=f"xt_{g}_{tt}")
                        r0 = (g * 4 + tt) * 128
                        # split each tile across both DMA rings so single
                        # tiles land ~2x faster (head latency) and the rings
                        # stay balanced
                        nc.sync.dma_start(
                            t[0:64, :], x_d.ap()[r0:r0 + 64, :])
                        nc.gpsimd.dma_start(
                            t[64:128, :], x_d.ap()[r0 + 64:r0 + 128, :])
                        xt.append(t)
                    xts[g] = xt
                    pos[g] = psc.tile([18, 512], F32, tag="po", name=f"po_{g}")
                pt = pxT.tile([128, 512], F32R, tag="pt", name=f"pt_{j}")
                for tt in range(4):
                    nc.tensor.transpose(
                        pt[:, tt * 128:(tt + 1) * 128],
                        xts[g][tt][:, k * 128:(k + 1) * 128],
                        idt[:],
                    )
                st = xtp.tile([128, 512], F32R, tag="st", name=f"st_{j}")
                if j % 2 == 0:
                    nc.vector.tensor_copy(st[:], pt[:])
                else:
                    nc.scalar.copy(st[:], pt[:])
                sts[j] = st
            jm = j - MM_LAG
            if jm >= 0:
                gm, km = jm // N_CHUNKS, jm % N_CHUNKS
                nc.tensor.matmul(
                    pos[gm][:], wTt[:, km * 18:(km + 1) * 18], sts.pop(jm)[:],
                    start=(km == 0), stop=(km == N_CHUNKS - 1),
                )
                if km == N_CHUNKS - 1:
                    emit_group_epilogue(gm)

        nc.sync.dma_start(oc_d.ap()[:], cnt[:])

    nc.compile()
    _cache["nc"] = nc
    return nc


def _run(in_maps, trace=False, tmpdir=None):
    from concourse import bass_utils
    if trace:
        bass_utils.upload_artifacts = lambda d: "local://" + d
    nc = _build()
    return bass_utils.run_bass_kernel_spmd(
        nc, in_maps, core_ids=list(range(N_CORES)), trace=trace, tmpdir=tmpdir
    )


def _make_in_maps(x, W_expert, W_group):
    x = np.ascontiguousarray(np.asarray(x, dtype=np.float32)).reshape(TOK, D)
    W_all = np.concatenate(
        [np.asarray(W_expert, np.float32), np.asarray(W_group, np.float32)], axis=0
    )
    # pack W^T (D, 18) into the SBUF tile layout (128, N_CHUNKS*18):
    # element (p, k*18+e) = W_all[e, k*128+p]
    wT = _round_f32r(
        np.ascontiguousarray(
            W_all.T.reshape(N_CHUNKS, 128, 18).transpose(1, 0, 2).reshape(128, -1)
        )
    )
    ident = np.eye(128, dtype=np.float32)
    import ml_dtypes
    ident_bf = np.eye(128, dtype=ml_dtypes.bfloat16)
    iota = np.tile(np.arange(16, dtype=np.float32), (128, 16))
    return [
        {"x": x[c * TPC:(c + 1) * TPC], "wT": wT, "ident": ident,
         "ident_bf": ident_bf, "iota": iota}
        for c in range(N_CORES)
    ]


def _gather(results):
    w_parts, i_parts = [], []
    counts = np.zeros(E, dtype=np.float64)
    for c in range(N_CORES):
        r = results[c]
        w_parts.append(
            r["out_w"].reshape(128, N_TILES, 6).transpose(1, 0, 2).reshape(TPC, 6)
        )
        i_parts.append(
            r["out_i"].reshape(128, N_TILES, 6).transpose(1, 0, 2).reshape(TPC, 6)
        )
        counts += r["out_c"].astype(np.float64).reshape(128, N_HALVES, E).sum(axis=(0, 1))
    weights = np.concatenate(w_parts, 0).reshape(B, T, 6).astype(np.float32)
    indices = np.concatenate(i_parts, 0).reshape(B, T, 6).astype(np.int32)
    counts[0] += 2.0 * TOK  # two zero pad slots per token
    actual = counts / counts.sum()
    aux = np.float32(0.01 * np.sum((1.0 / E) * (np.log(1.0 / E) - np.log(actual))))
    return weights, indices, aux


def kernel(x, W_expert, W_group):
    in_maps = _make_in_maps(x, W_expert, W_group)
    res = _run(in_maps)
    return _gather(res.results)


if __name__ == "__main__":
    rng = np.random.default_rng(1)
    x = rng.normal(size=(B, T, D)).astype(np.float32)
    We = (rng.normal(size=(E, D)) * D ** -0.5).astype(np.float32)
    Wg = (rng.normal(size=(2, D)) * D ** -0.5).astype(np.float32)
    w, i, aux = kernel(x, We, Wg)
    print("weights", w.shape, w.dtype, "indices", i.shape, i.dtype, "aux", aux)


# revision 37
# speedup vs baseline: 1.1232x; 1.1232x over previous
"""ExpertGroupRouter MoE routing kernel for 8 TRN2 NeuronCores (Bass/Tile).

Strategy (data-parallel over tokens):
  - Flatten x to (16384, 2048) tokens; shard 2048 contiguous tokens per core.
  - Per core: stream x in 16 tiles of (128 tok, 2048 d). PE-transpose each
    128x128 block to build xT (d-major) chunks, copy PSUM->SBUF (DVE/ACT
    alternating), then PE matmul with the replicated 18-col weight matrix
    W^T (W_expert ++ W_group) to get scoresT (18, tok) with f32 PSUM
    accumulation over the 16 d-chunks.  float32r (11-bit-mantissa fp32) is
    used on the PE for 4x matmul and ~2.5x transpose throughput; the
    routing decisions stay f32.
  - scoresT is PE-transposed back to token-major (128, 18) tiles and the
    full routing logic (grouped softmax top-1/top-1/top-2, sigmoid gates,
    weight normalization, expert bincount) runs batched on DVE/ACT, split
    into two halves so it overlaps the score streaming.
  - x loads alternate between the SP and ACT hardware DGE rings to use
    both HWDGE queues.
  - Host gathers the 8 shards, sums the per-core/per-partition expert
    counts and computes the scalar KL aux loss from the 16 totals.
"""
import sys

if "/opt/trn_rl_repo" not in sys.path:
    sys.path.insert(0, "/opt/trn_rl_repo")

import numpy as np

B, T, D, E = 4, 4096, 2048, 16
N_CORES = 8
TOK = B * T                  # 16384 tokens
TPC = TOK // N_CORES         # 2048 tokens per core
N_TILES = TPC // 128         # 16 token tiles per core
N_CHUNKS = D // 128          # 16 d-chunks
GROUPS = N_TILES // 4        # 4 groups of 512 tokens
BIG = 65536.0
THR = 0.15
N_HALVES = 2                 # routing passes (overlap with streaming)

_cache = {}


def _round_f32r(a: np.ndarray) -> np.ndarray:
    """Round f32 bits to the PE's fp32r format (RNE to 11 mantissa bits)."""
    u = a.astype(np.float32).view(np.uint32).astype(np.uint64)
    u = (u + 0x7FF + ((u >> 12) & 1)) & 0xFFFFF000
    return (u & 0xFFFFFFFF).astype(np.uint32).view(np.float32)


def _build():
    if "nc" in _cache:
        return _cache["nc"]
    import concourse.bass as bass
    import concourse.tile as tile
    from concourse import mybir, bacc

    F32, F32R, I32 = mybir.dt.float32, mybir.dt.float32r, mybir.dt.int32
    AX = mybir.AxisListType
    OP = mybir.AluOpType
    ACTF = mybir.ActivationFunctionType

    nc = bacc.Bacc("TRN2", target_bir_lowering=False, debug=False)
    x_d = nc.dram_tensor("x", [TPC, D], F32R, kind="ExternalInput")
    # wT packed host-side into the SBUF layout: (128, N_CHUNKS*18)
    wT_d = nc.dram_tensor("wT", [128, N_CHUNKS * 18], F32R, kind="ExternalInput")
    id_d = nc.dram_tensor("ident", [128, 128], F32R, kind="ExternalInput")
    idb_d = nc.dram_tensor("ident_bf", [128, 128], mybir.dt.bfloat16,
                           kind="ExternalInput")
    iota_d = nc.dram_tensor("iota", [128, 16 * 16], F32, kind="ExternalInput")
    ow_d = nc.dram_tensor("out_w", [128, N_TILES * 6], F32, kind="ExternalOutput")
    oi_d = nc.dram_tensor("out_i", [128, N_TILES * 6], I32, kind="ExternalOutput")
    oc_d = nc.dram_tensor("out_c", [128, N_HALVES * E], F32, kind="ExternalOutput")

    import contextlib

    with tile.TileContext(nc) as tc, contextlib.ExitStack() as ctx:
        const = ctx.enter_context(tc.tile_pool(name="const", bufs=1))
        xpool = ctx.enter_context(tc.tile_pool(name="xpool", bufs=12))
        xtp = ctx.enter_context(tc.tile_pool(name="xtp", bufs=4))
        pxT = ctx.enter_context(tc.tile_pool(name="pxT", bufs=3, space="PSUM"))
        psc = ctx.enter_context(tc.tile_pool(name="psc", bufs=2, space="PSUM"))
        pbt = ctx.enter_context(tc.tile_pool(name="pbt", bufs=2, space="PSUM"))
        sc_sb = ctx.enter_context(tc.tile_pool(name="sc_sb", bufs=2))
        rt = ctx.enter_context(tc.tile_pool(name="rt", bufs=1))
        scr = ctx.enter_context(tc.tile_pool(name="scr", bufs=2))

        # ---- constants (identity first — transposes need it; the rest
        # is loaded on the gpsimd ring so the sync ring starts on x) ----
        idt = const.tile([128, 128], F32R)
        nc.sync.dma_start(idt[:], id_d.ap()[:])
        wTt = const.tile([128, N_CHUNKS * 18], F32R)
        nc.gpsimd.dma_start(wTt[:], wT_d.ap()[:])
        iot = const.tile([128, 16 * 16], F32)
        nc.gpsimd.dma_start(iot[:], iota_d.ap()[:])

        # persistent tiles
        S_all = rt.tile([128, N_TILES * 18], F32)   # token-major scores
        ow_sb = rt.tile([128, N_TILES * 6], F32)
        oi_sb = rt.tile([128, N_TILES * 6], I32)
        cnt = rt.tile([128, N_HALVES * E], F32)

        io3 = iot[:].rearrange("p (t e) -> p t e", t=16)

        def slot(view3, j):
            return view3[:, :, j:j + 1].rearrange("p t a -> p (t a)")

        # ---------------- routing block over tiles [ts, ts+tn) ----------
        def routing_block(h, ts, tn):
            S = S_all[:, ts * 18:(ts + tn) * 18].rearrange(
                "p (t e) -> p t e", t=tn
            )
            ioA = io3[:, 0:tn, 0:8]
            ioB = io3[:, 0:tn, 8:12]
            ioC = io3[:, 0:tn, 12:16]

            def bcast(t2d, w):
                return t2d[:].to_broadcast([128, tn, w])

            sa, sb4, sc4 = S[:, :, 0:8], S[:, :, 8:12], S[:, :, 12:16]
            gsc = S[:, :, 16:18]

            def tl(name, wdt=1, dt=F32):
                return scr.tile([128, tn * wdt], dt, tag=name, name=f"{name}_h{h}")

            m_a, m_b, m_c = tl("m_a"), tl("m_b"), tl("m_c")
            nc.vector.reduce_max(m_a[:], sa, axis=AX.X)
            nc.vector.reduce_max(m_b[:], sb4, axis=AX.X)
            nc.vector.reduce_max(m_c[:], sc4, axis=AX.X)

            E_all = tl("E_all", 16)
            E3 = E_all[:].rearrange("p (t e) -> p t e", t=tn)
            nc.scalar.activation(E3, S[:, :, 0:16], ACTF.Exp)
            sum_a, sum_b, sum_c = tl("sum_a"), tl("sum_b"), tl("sum_c")
            nc.vector.reduce_sum(sum_a[:], E3[:, :, 0:8], axis=AX.X)
            nc.vector.reduce_sum(sum_b[:], E3[:, :, 8:12], axis=AX.X)
            nc.vector.reduce_sum(sum_c[:], E3[:, :, 12:16], axis=AX.X)

            em_a, em_b, em_c = tl("em_a"), tl("em_b"), tl("em_c")
            nc.scalar.activation(em_a[:], m_a[:], ACTF.Exp)
            nc.scalar.activation(em_b[:], m_b[:], ACTF.Exp)
            nc.scalar.activation(em_c[:], m_c[:], ACTF.Exp)

            ra, rb, rc = tl("ra"), tl("rb"), tl("rc")
            nc.vector.reciprocal(ra[:], sum_a[:])
            nc.vector.reciprocal(rb[:], sum_b[:])
            nc.vector.reciprocal(rc[:], sum_c[:])

            gs = tl("gs", 2)
            g3 = gs[:].rearrange("p (t g) -> p t g", t=tn)
            nc.scalar.activation(g3, gsc, ACTF.Sigmoid)

            gm0, gm1, msk = tl("gm0"), tl("gm1"), tl("msk")
            g0 = g3[:, :, 0:1].rearrange("p t a -> p (t a)")
            g1 = g3[:, :, 1:2].rearrange("p t a -> p (t a)")
            nc.vector.tensor_scalar(msk[:], g0, THR, None, op0=OP.is_gt)
            nc.vector.tensor_tensor(gm0[:], msk[:], g0, op=OP.mult)
            nc.vector.tensor_scalar(msk[:], g1, THR, None, op0=OP.is_gt)
            nc.vector.tensor_tensor(gm1[:], msk[:], g1, op=OP.mult)

            scr8 = tl("scr8", 8)
            s8 = scr8[:].rearrange("p (t e) -> p t e", t=tn)
            scr4 = tl("scr4", 4)
            s4 = scr4[:].rearrange("p (t e) -> p t e", t=tn)

            def argmax_idx(out, s_view, m_t, io_view, w, sc3):
                nc.vector.tensor_tensor(sc3, s_view, bcast(m_t, w), op=OP.is_lt)
                nc.vector.scalar_tensor_tensor(
                    sc3, sc3, BIG, io_view, op0=OP.mult, op1=OP.add
                )
                nc.vector.tensor_reduce(out, sc3, axis=AX.X, op=OP.min)

            idx_a, idx_b = tl("idx_a"), tl("idx_b")
            idx_c1, idx_c2 = tl("idx_c1"), tl("idx_c2")
            argmax_idx(idx_a[:], sa, m_a, ioA, 8, s8)
            argmax_idx(idx_b[:], sb4, m_b, ioB, 4, s4)
            argmax_idx(idx_c1[:], sc4, m_c, ioC, 4, s4)

            sc_m = tl("sc_m", 4)
            sm4 = sc_m[:].rearrange("p (t e) -> p t e", t=tn)
            nc.vector.tensor_tensor(s4, ioC, bcast(idx_c1, 4), op=OP.is_equal)
            nc.vector.scalar_tensor_tensor(sm4, s4, -BIG, sc4, op0=OP.mult, op1=OP.add)
            m_c2 = tl("m_c2")
            nc.vector.reduce_max(m_c2[:], sm4, axis=AX.X)
            argmax_idx(idx_c2[:], sm4, m_c2, ioC, 4, s4)
            em_c2 = tl("em_c2")
            nc.scalar.activation(em_c2[:], m_c2[:], ACTF.Exp)

            # weights
            W_raw = tl("W_raw", 6)
            W3 = W_raw[:].rearrange("p (t s) -> p t s", t=tn)
            nc.vector.memset(W_raw[:], 0.0)
            tb = tl("tb")
            nc.vector.tensor_tensor(slot(W3, 0), em_a[:], ra[:], op=OP.mult)
            nc.vector.tensor_tensor(tb[:], em_b[:], rb[:], op=OP.mult)
            nc.vector.tensor_tensor(slot(W3, 1), tb[:], gm0[:], op=OP.mult)
            nc.vector.tensor_tensor(tb[:], em_c[:], rc[:], op=OP.mult)
            nc.vector.tensor_tensor(slot(W3, 2), tb[:], gm1[:], op=OP.mult)
            nc.vector.tensor_tensor(tb[:], em_c2[:], rc[:], op=OP.mult)
            nc.vector.tensor_tensor(slot(W3, 3), tb[:], gm1[:], op=OP.mult)

            sum_w, winv = tl("sum_w"), tl("winv")
            nc.vector.reduce_sum(sum_w[:], W3[:, :, 0:4], axis=AX.X)
            nc.vector.tensor_scalar(sum_w[:], sum_w[:], 1e-8, None, op0=OP.add)
            nc.vector.reciprocal(winv[:], sum_w[:])

            ow3 = ow_sb[:, ts * 6:(ts + tn) * 6].rearrange("p (t s) -> p t s", t=tn)
            nc.vector.tensor_tensor(ow3, W3, bcast(winv, 6), op=OP.mult)
            nc.sync.dma_start(
                ow_d.ap()[:, ts * 6:(ts + tn) * 6], ow_sb[:, ts * 6:(ts + tn) * 6]
            )

            # indices
            I_f = tl("I_f", 6)
            I3 = I_f[:].rearrange("p (t s) -> p t s", t=tn)
            nc.vector.memset(I_f[:], 0.0)
            nc.vector.tensor_copy(slot(I3, 0), idx_a[:])
            nc.vector.tensor_copy(slot(I3, 1), idx_b[:])
            nc.vector.tensor_copy(slot(I3, 2), idx_c1[:])
            nc.vector.tensor_copy(slot(I3, 3), idx_c2[:])
            oi_slice = oi_sb[:, ts * 6:(ts + tn) * 6]
            nc.vector.tensor_copy(oi_slice, I_f[:])
            nc.sync.dma_start(oi_d.ap()[:, ts * 6:(ts + tn) * 6], oi_slice)

            # counts
            cdummy = tl("cdummy", 4)
            cd3 = cdummy[:].rearrange("p (t s) -> p t s", t=tn)
            for e in range(E):
                nc.vector.tensor_scalar(
                    cd3, I3[:, :, 0:4], float(e), 0.0, op0=OP.is_equal, op1=OP.add,
                    accum_out=cnt[:, h * E + e:h * E + e + 1],
                )

        # ---------------- main streaming loop (software-pipelined) ------
        # Per global chunk slot j (group g = j//16, chunk k = j%16):
        #   - issue the group's x-tile DMAs at its first slot
        #   - emit 4 PE transposes for chunk j and the PSUM->SBUF copy
        #   - emit the score matmul for chunk j-MM_LAG (so the PE, which
        #     executes in order, never waits on an in-flight copy)
        MM_LAG = 2
        N_SLOTS = GROUPS * N_CHUNKS
        xts = {}         # group -> list of 4 x tiles
        pos = {}         # group -> psum scores tile
        sts = {}         # global chunk j -> xT sbuf tile

        def emit_group_epilogue(g):
            # scoresT (18, 512) -> token-major (128, 18) tiles
            sct = sc_sb.tile([18, 512], F32, tag="sct", name=f"sct_{g}")
            nc.scalar.copy(sct[:], pos[g][:])
            for tt in range(4):
                pb = pbt.tile([128, 18], F32, tag="pb", name=f"pb_{g}_{tt}")
                nc.tensor.transpose(
                    pb[:], sct[:, tt * 128:(tt + 1) * 128],
                    idt[0:18, 0:18].bitcast(F32),
                )
                dst = S_all[:, (g * 4 + tt) * 18:(g * 4 + tt + 1) * 18]
                nc.scalar.copy(dst, pb[:])
            if g == GROUPS // 2 - 1:
                routing_block(0, 0, N_TILES // 2)
            elif g == GROUPS - 1:
                routing_block(1, N_TILES // 2, N_TILES // 2)

        for j in range(N_SLOTS + MM_LAG):
            g, k = j // N_CHUNKS, j % N_CHUNKS
            if j < N_SLOTS:
                if k == 0:
                    xt = []
                    for tt in range(4):
                        t = xpool.tile([128, D], F32R, tag="xt",
                                       name=f"xt_{g}_{tt}")
                        r0 = (g * 4 + tt) * 128
                        # split each tile by free-dim across both DMA rings
                        # (keeps 128 partitions per transfer for full port
                        # coverage) so single tiles land ~2x faster
                        nc.sync.dma_start(
                            t[:, 0:D // 2], x_d.ap()[r0:r0 + 128, 0:D // 2])
                        nc.gpsimd.dma_start(
                            t[:, D // 2:D], x_d.ap()[r0:r0 + 128, D // 2:D])
                        xt.append(t)
                    xts[g] = xt
                    pos[g] = psc.tile([18, 512], F32, tag="po", name=f"po_{g}")
                pt = pxT.tile([128, 512], F32R, tag="pt", name=f"pt_{j}")
                for tt in range(4):
                    nc.tensor.transpose(
                        pt[:, tt * 128:(tt + 1) * 128],
                        xts[g][tt][:, k * 128:(k + 1) * 128],
                        idt[:],
                    )
                st = xtp.tile([128, 512], F32R, tag="st", name=f"st_{j}")
                if j % 2 == 0:
                    nc.vector.tensor_copy(st[:], pt[:])
                else:
                    nc.scalar.copy(st[:], pt[:])
                sts[j] = st
            jm = j - MM_LAG
            if jm >= 0:
                gm, km = jm // N_CHUNKS, jm % N_CHUNKS
                nc.tensor.matmul(
                    pos[gm][:], wTt[:, km * 18:(km + 1) * 18], sts.pop(jm)[:],
                    start=(km == 0), stop=(km == N_CHUNKS - 1),
                )
                if km == N_CHUNKS - 1:
                    emit_group_epilogue(gm)

        nc.sync.dma_start(oc_d.ap()[:], cnt[:])

    nc.compile()
    _cache["nc"] = nc
    return nc


def _run(in_maps, trace=False, tmpdir=None):
    from concourse import bass_utils
    if trace:
        bass_utils.upload_artifacts = lambda d: "local://" + d
    nc = _build()
    return bass_utils.run_bass_kernel_spmd(
        nc, in_maps, core_ids=list(range(N_CORES)), trace=trace, tmpdir=tmpdir
    )


def _make_in_maps(x, W_expert, W_group):
    x = np.ascontiguousarray(np.asarray(x, dtype=np.float32)).reshape(TOK, D)
    W_all = np.concatenate(
        [np.asarray(W_expert, np.float32), np.asarray(W_group, np.float32)], axis=0
    )
    # pack W^T (D, 18) into the SBUF tile layout (128, N_CHUNKS*18):
    # element (p, k*18+e) = W_all[e, k*128+p]
    wT = _round_f32r(
        np.ascontiguousarray(
            W_all.T.reshape(N_CHUNKS, 128, 18).transpose(1, 0, 2).reshape(128, -1)
        )
    )
    ident = np.eye(128, dtype=np.float32)
    import ml_dtypes
    ident_bf = np.eye(128, dtype=ml_dtypes.bfloat16)
    iota = np.tile(np.arange(16, dtype=np.float32), (128, 16))
    return [
        {"x": x[c * TPC:(c + 1) * TPC], "wT": wT, "ident": ident,
         "ident_bf": ident_bf, "iota": iota}
        for c in range(N_CORES)
    ]


def _gather(results):
    w_parts, i_parts = [], []
    counts = np.zeros(E, dtype=np.float64)
    for c in range(N_CORES):
        r = results[c]
        w_parts.append(
            r["out_w"].reshape(128, N_TILES, 6).transpose(1, 0, 2).reshape(TPC, 6)
        )
        i_parts.append(
            r["out_i"].reshape(128, N_TILES, 6).transpose(1, 0, 2).reshape(TPC, 6)
        )
        counts += r["out_c"].astype(np.float64).reshape(128, N_HALVES, E).sum(axis=(0, 1))
    weights = np.concatenate(w_parts, 0).reshape(B, T, 6).astype(np.float32)
    indices = np.concatenate(i_parts, 0).reshape(B, T, 6).astype(np.int32)
    counts[0] += 2.0 * TOK  # two zero pad slots per token
    actual = counts / counts.sum()
    aux = np.float32(0.01 * np.sum((1.0 / E) * (np.log(1.0 / E) - np.log(actual))))
    return weights, indices, aux


def kernel(x, W_expert, W_group):
    in_maps = _make_in_maps(x, W_expert, W_group)
    res = _run(in_maps)
    return _gather(res.results)


if __name__ == "__main__":
    rng = np.random.default_rng(1)
    x = rng.normal(size=(B, T, D)).astype(np.float32)
    We = (rng.normal(size=(E, D)) * D ** -0.5).astype(np.float32)
    Wg = (rng.normal(size=(2, D)) * D ** -0.5).astype(np.float32)
    w, i, aux = kernel(x, We, Wg)
    print("weights", w.shape, w.dtype, "indices", i.shape, i.dtype, "aux", aux)


# revision 40
# speedup vs baseline: 1.1687x; 1.0405x over previous
"""ExpertGroupRouter MoE routing kernel for 8 TRN2 NeuronCores (Bass/Tile).

Strategy (data-parallel over tokens):
  - Flatten x to (16384, 2048) tokens; shard 2048 contiguous tokens per core.
  - Per core: stream x in 16 tiles of (128 tok, 2048 d). PE-transpose each
    128x128 block to build xT (d-major) chunks, copy PSUM->SBUF (DVE/ACT
    alternating), then PE matmul with the replicated 18-col weight matrix
    W^T (W_expert ++ W_group) to get scoresT (18, tok) with f32 PSUM
    accumulation over the 16 d-chunks.  float32r (11-bit-mantissa fp32) is
    used on the PE for 4x matmul and ~2.5x transpose throughput; the
    routing decisions stay f32.
  - scoresT is PE-transposed back to token-major (128, 18) tiles and the
    full routing logic (grouped softmax top-1/top-1/top-2, sigmoid gates,
    weight normalization, expert bincount) runs batched on DVE/ACT, split
    into two halves so it overlaps the score streaming.
  - x loads alternate between the SP and ACT hardware DGE rings to use
    both HWDGE queues.
  - Host gathers the 8 shards, sums the per-core/per-partition expert
    counts and computes the scalar KL aux loss from the 16 totals.
"""
import sys

if "/opt/trn_rl_repo" not in sys.path:
    sys.path.insert(0, "/opt/trn_rl_repo")

import numpy as np

B, T, D, E = 4, 4096, 2048, 16
N_CORES = 8
TOK = B * T                  # 16384 tokens
TPC = TOK // N_CORES         # 2048 tokens per core
N_TILES = TPC // 128         # 16 token tiles per core
N_CHUNKS = D // 128          # 16 d-chunks
GROUPS = N_TILES // 4        # 4 groups of 512 tokens
BIG = 65536.0
THR = 0.15
N_HALVES = 2                 # routing passes (overlap with streaming)

_cache = {}


def _round_f32r(a: np.ndarray) -> np.ndarray:
    """Round f32 bits to the PE's fp32r format (RNE to 11 mantissa bits)."""
    u = a.astype(np.float32).view(np.uint32).astype(np.uint64)
    u = (u + 0x7FF + ((u >> 12) & 1)) & 0xFFFFF000
    return (u & 0xFFFFFFFF).astype(np.uint32).view(np.float32)


def _build():
    if "nc" in _cache:
        return _cache["nc"]
    import concourse.bass as bass
    import concourse.tile as tile
    from concourse import mybir, bacc

    F32, F32R, I32 = mybir.dt.float32, mybir.dt.float32r, mybir.dt.int32
    AX = mybir.AxisListType
    OP = mybir.AluOpType
    ACTF = mybir.ActivationFunctionType

    nc = bacc.Bacc("TRN2", target_bir_lowering=False, debug=False)
    x_d = nc.dram_tensor("x", [TPC, D], F32R, kind="ExternalInput")
    # wT packed host-side into the SBUF layout: (128, N_CHUNKS*18)
    wT_d = nc.dram_tensor("wT", [128, N_CHUNKS * 18], F32R, kind="ExternalInput")
    id_d = nc.dram_tensor("ident", [128, 128], F32R, kind="ExternalInput")
    idb_d = nc.dram_tensor("ident_bf", [128, 128], mybir.dt.bfloat16,
                           kind="ExternalInput")
    iota_d = nc.dram_tensor("iota", [128, 16 * 16], F32, kind="ExternalInput")
    ow_d = nc.dram_tensor("out_w", [128, N_TILES * 6], F32, kind="ExternalOutput")
    oi_d = nc.dram_tensor("out_i", [128, N_TILES * 6], I32, kind="ExternalOutput")
    oc_d = nc.dram_tensor("out_c", [128, N_HALVES * E], F32, kind="ExternalOutput")

    import contextlib

    with tile.TileContext(nc) as tc, contextlib.ExitStack() as ctx:
        const = ctx.enter_context(tc.tile_pool(name="const", bufs=1))
        xpool = ctx.enter_context(tc.tile_pool(name="xpool", bufs=12))
        xtp = ctx.enter_context(tc.tile_pool(name="xtp", bufs=4))
        pxT = ctx.enter_context(tc.tile_pool(name="pxT", bufs=3, space="PSUM"))
        psc = ctx.enter_context(tc.tile_pool(name="psc", bufs=2, space="PSUM"))
        pbt = ctx.enter_context(tc.tile_pool(name="pbt", bufs=2, space="PSUM"))
        sc_sb = ctx.enter_context(tc.tile_pool(name="sc_sb", bufs=2))
        rt = ctx.enter_context(tc.tile_pool(name="rt", bufs=1))
        scr = ctx.enter_context(tc.tile_pool(name="scr", bufs=2))

        # ---- constants (identity first — transposes need it; the rest
        # is loaded on the gpsimd ring so the sync ring starts on x) ----
        idt = const.tile([128, 128], F32R)
        nc.sync.dma_start(idt[:], id_d.ap()[:])
        wTt = const.tile([128, N_CHUNKS * 18], F32R)
        nc.gpsimd.dma_start(wTt[:], wT_d.ap()[:])
        iot = const.tile([128, 16 * 16], F32)
        nc.gpsimd.dma_start(iot[:], iota_d.ap()[:])

        # persistent tiles
        S_all = rt.tile([128, N_TILES * 18], F32)   # token-major scores
        ow_sb = rt.tile([128, N_TILES * 6], F32)
        oi_sb = rt.tile([128, N_TILES * 6], I32)
        cnt = rt.tile([128, N_HALVES * E], F32)

        io3 = iot[:].rearrange("p (t e) -> p t e", t=16)

        def slot(view3, j):
            return view3[:, :, j:j + 1].rearrange("p t a -> p (t a)")

        # ---------------- routing block over tiles [ts, ts+tn) ----------
        def routing_block(h, ts, tn):
            S = S_all[:, ts * 18:(ts + tn) * 18].rearrange(
                "p (t e) -> p t e", t=tn
            )
            ioA = io3[:, 0:tn, 0:8]
            ioB = io3[:, 0:tn, 8:12]
            ioC = io3[:, 0:tn, 12:16]

            def bcast(t2d, w):
                return t2d[:].to_broadcast([128, tn, w])

            sa, sb4, sc4 = S[:, :, 0:8], S[:, :, 8:12], S[:, :, 12:16]
            gsc = S[:, :, 16:18]

            def tl(name, wdt=1, dt=F32):
                return scr.tile([128, tn * wdt], dt, tag=name, name=f"{name}_h{h}")

            m_a, m_b, m_c = tl("m_a"), tl("m_b"), tl("m_c")
            nc.vector.reduce_max(m_a[:], sa, axis=AX.X)
            nc.vector.reduce_max(m_b[:], sb4, axis=AX.X)
            nc.vector.reduce_max(m_c[:], sc4, axis=AX.X)

            E_all = tl("E_all", 16)
            E3 = E_all[:].rearrange("p (t e) -> p t e", t=tn)
            nc.scalar.activation(E3, S[:, :, 0:16], ACTF.Exp)
            sum_a, sum_b, sum_c = tl("sum_a"), tl("sum_b"), tl("sum_c")
            nc.vector.reduce_sum(sum_a[:], E3[:, :, 0:8], axis=AX.X)
            nc.vector.reduce_sum(sum_b[:], E3[:, :, 8:12], axis=AX.X)
            nc.vector.reduce_sum(sum_c[:], E3[:, :, 12:16], axis=AX.X)

            em_a, em_b, em_c = tl("em_a"), tl("em_b"), tl("em_c")
            nc.scalar.activation(em_a[:], m_a[:], ACTF.Exp)
            nc.scalar.activation(em_b[:], m_b[:], ACTF.Exp)
            nc.scalar.activation(em_c[:], m_c[:], ACTF.Exp)

            ra, rb, rc = tl("ra"), tl("rb"), tl("rc")
            nc.vector.reciprocal(ra[:], sum_a[:])
            nc.vector.reciprocal(rb[:], sum_b[:])
            nc.vector.reciprocal(rc[:], sum_c[:])

            # sigmoid via exp (reuses the Exp ACT table: no table reload):
            # g = 1 / (1 + exp(-s))
            gs = tl("gs", 2)
            g3 = gs[:].rearrange("p (t g) -> p t g", t=tn)
            nc.scalar.activation(g3, gsc, ACTF.Exp, scale=-1.0)
            nc.vector.tensor_scalar(gs[:], gs[:], 1.0, None, op0=OP.add)
            nc.vector.reciprocal(gs[:], gs[:])

            gm0, gm1, msk = tl("gm0"), tl("gm1"), tl("msk")
            g0 = g3[:, :, 0:1].rearrange("p t a -> p (t a)")
            g1 = g3[:, :, 1:2].rearrange("p t a -> p (t a)")
            nc.vector.tensor_scalar(msk[:], g0, THR, None, op0=OP.is_gt)
            nc.vector.tensor_tensor(gm0[:], msk[:], g0, op=OP.mult)
            nc.vector.tensor_scalar(msk[:], g1, THR, None, op0=OP.is_gt)
            nc.vector.tensor_tensor(gm1[:], msk[:], g1, op=OP.mult)

            scr8 = tl("scr8", 8)
            s8 = scr8[:].rearrange("p (t e) -> p t e", t=tn)
            scr4 = tl("scr4", 4)
            s4 = scr4[:].rearrange("p (t e) -> p t e", t=tn)

            def argmax_idx(out, s_view, m_t, io_view, w, sc3):
                nc.vector.tensor_tensor(sc3, s_view, bcast(m_t, w), op=OP.is_lt)
                nc.vector.scalar_tensor_tensor(
                    sc3, sc3, BIG, io_view, op0=OP.mult, op1=OP.add
                )
                nc.vector.tensor_reduce(out, sc3, axis=AX.X, op=OP.min)

            idx_a, idx_b = tl("idx_a"), tl("idx_b")
            idx_c1, idx_c2 = tl("idx_c1"), tl("idx_c2")
            argmax_idx(idx_a[:], sa, m_a, ioA, 8, s8)
            argmax_idx(idx_b[:], sb4, m_b, ioB, 4, s4)
            argmax_idx(idx_c1[:], sc4, m_c, ioC, 4, s4)

            sc_m = tl("sc_m", 4)
            sm4 = sc_m[:].rearrange("p (t e) -> p t e", t=tn)
            nc.vector.tensor_tensor(s4, ioC, bcast(idx_c1, 4), op=OP.is_equal)
            nc.vector.scalar_tensor_tensor(sm4, s4, -BIG, sc4, op0=OP.mult, op1=OP.add)
            m_c2 = tl("m_c2")
            nc.vector.reduce_max(m_c2[:], sm4, axis=AX.X)
            argmax_idx(idx_c2[:], sm4, m_c2, ioC, 4, s4)
            em_c2 = tl("em_c2")
            nc.scalar.activation(em_c2[:], m_c2[:], ACTF.Exp)

            # weights
            W_raw = tl("W_raw", 6)
            W3 = W_raw[:].rearrange("p (t s) -> p t s", t=tn)
            nc.vector.memset(W_raw[:], 0.0)
            tb = tl("tb")
            nc.vector.tensor_tensor(slot(W3, 0), em_a[:], ra[:], op=OP.mult)
            nc.vector.tensor_tensor(tb[:], em_b[:], rb[:], op=OP.mult)
            nc.vector.tensor_tensor(slot(W3, 1), tb[:], gm0[:], op=OP.mult)
            nc.vector.tensor_tensor(tb[:], em_c[:], rc[:], op=OP.mult)
            nc.vector.tensor_tensor(slot(W3, 2), tb[:], gm1[:], op=OP.mult)
            nc.vector.tensor_tensor(tb[:], em_c2[:], rc[:], op=OP.mult)
            nc.vector.tensor_tensor(slot(W3, 3), tb[:], gm1[:], op=OP.mult)

            sum_w, winv = tl("sum_w"), tl("winv")
            nc.vector.reduce_sum(sum_w[:], W3[:, :, 0:4], axis=AX.X)
            nc.vector.tensor_scalar(sum_w[:], sum_w[:], 1e-8, None, op0=OP.add)
            nc.vector.reciprocal(winv[:], sum_w[:])

            ow3 = ow_sb[:, ts * 6:(ts + tn) * 6].rearrange("p (t s) -> p t s", t=tn)
            nc.vector.tensor_tensor(ow3, W3, bcast(winv, 6), op=OP.mult)
            nc.sync.dma_start(
                ow_d.ap()[:, ts * 6:(ts + tn) * 6], ow_sb[:, ts * 6:(ts + tn) * 6]
            )

            # indices
            I_f = tl("I_f", 6)
            I3 = I_f[:].rearrange("p (t s) -> p t s", t=tn)
            nc.vector.memset(I_f[:], 0.0)
            nc.vector.tensor_copy(slot(I3, 0), idx_a[:])
            nc.vector.tensor_copy(slot(I3, 1), idx_b[:])
            nc.vector.tensor_copy(slot(I3, 2), idx_c1[:])
            nc.vector.tensor_copy(slot(I3, 3), idx_c2[:])
            oi_slice = oi_sb[:, ts * 6:(ts + tn) * 6]
            nc.vector.tensor_copy(oi_slice, I_f[:])
            nc.sync.dma_start(oi_d.ap()[:, ts * 6:(ts + tn) * 6], oi_slice)

            # counts
            cdummy = tl("cdummy", 4)
            cd3 = cdummy[:].rearrange("p (t s) -> p t s", t=tn)
            for e in range(E):
                nc.vector.tensor_scalar(
                    cd3, I3[:, :, 0:4], float(e), 0.0, op0=OP.is_equal, op1=OP.add,
                    accum_out=cnt[:, h * E + e:h * E + e + 1],
                )

        # ---------------- main streaming loop (software-pipelined) ------
        # Per global chunk slot j (group g = j//16, chunk k = j%16):
        #   - issue the group's x-tile DMAs at its first slot
        #   - emit 4 PE transposes for chunk j and the PSUM->SBUF copy
        #   - emit the score matmul for chunk j-MM_LAG (so the PE, which
        #     executes in order, never waits on an in-flight copy)
        MM_LAG = 2
        N_SLOTS = GROUPS * N_CHUNKS
        xts = {}         # group -> list of 4 x tiles
        pos = {}         # group -> psum scores tile
        sts = {}         # global chunk j -> xT sbuf tile

        scts = {}
        ROUTE_AFTER = {1: (0, 0, 8), 2: (1, 8, 4), 3: (2, 12, 4)}

        def emit_group_scT(g):
            # scoresT PSUM -> SBUF (frees the po accumulation bank)
            sct = sc_sb.tile([18, 512], F32, tag="sct", name=f"sct_{g}")
            nc.scalar.copy(sct[:], pos[g][:])
            scts[g] = sct

        def emit_group_epilogue(g):
            # token-major back-transposes; emitted a few slots into the
            # next group so the in-order PE never waits on the sct copy
            sct = scts.pop(g)
            for tt in range(4):
                pb = pbt.tile([128, 18], F32, tag="pb", name=f"pb_{g}_{tt}")
                nc.tensor.transpose(
                    pb[:], sct[:, tt * 128:(tt + 1) * 128],
                    idt[0:18, 0:18].bitcast(F32),
                )
                dst = S_all[:, (g * 4 + tt) * 18:(g * 4 + tt + 1) * 18]
                nc.scalar.copy(dst, pb[:])
            if g in ROUTE_AFTER:
                h, rts, rtn = ROUTE_AFTER[g]
                if h < N_HALVES - 1:
                    with tc.high_priority(offset=-600):
                        routing_block(h, rts, rtn)
                else:
                    routing_block(h, rts, rtn)

        for j in range(N_SLOTS + MM_LAG):
            g, k = j // N_CHUNKS, j % N_CHUNKS
            if j < N_SLOTS:
                if k == 0:
                    xt = []
                    for tt in range(4):
                        t = xpool.tile([128, D], F32R, tag="xt",
                                       name=f"xt_{g}_{tt}")
                        r0 = (g * 4 + tt) * 128
                        # split each tile by free-dim across both DMA rings
                        # (keeps 128 partitions per transfer for full port
                        # coverage) so single tiles land ~2x faster
                        nc.sync.dma_start(
                            t[:, 0:D // 2], x_d.ap()[r0:r0 + 128, 0:D // 2])
                        nc.gpsimd.dma_start(
                            t[:, D // 2:D], x_d.ap()[r0:r0 + 128, D // 2:D])
                        xt.append(t)
                    xts[g] = xt
                    pos[g] = psc.tile([18, 512], F32, tag="po", name=f"po_{g}")
                pt = pxT.tile([128, 512], F32R, tag="pt", name=f"pt_{j}")
                for tt in range(4):
                    nc.tensor.transpose(
                        pt[:, tt * 128:(tt + 1) * 128],
                        xts[g][tt][:, k * 128:(k + 1) * 128],
                        idt[:],
                    )
                st = xtp.tile([128, 512], F32R, tag="st", name=f"st_{j}")
                if j % 2 == 0:
                    nc.vector.tensor_copy(st[:], pt[:])
                else:
                    nc.scalar.copy(st[:], pt[:])
                sts[j] = st
            jm = j - MM_LAG
            if jm >= 0:
                gm, km = jm // N_CHUNKS, jm % N_CHUNKS
                nc.tensor.matmul(
                    pos[gm][:], wTt[:, km * 18:(km + 1) * 18], sts.pop(jm)[:],
                    start=(km == 0), stop=(km == N_CHUNKS - 1),
                )
                if km == N_CHUNKS - 1:
                    emit_group_scT(gm)
                elif km == 2 and gm > 0:
                    emit_group_epilogue(gm - 1)
        emit_group_epilogue(GROUPS - 1)

        nc.sync.dma_start(oc_d.ap()[:], cnt[:])

    nc.compile()
    _cache["nc"] = nc
    return nc


def _run(in_maps, trace=False, tmpdir=None):
    from concourse import bass_utils
    if trace:
        bass_utils.upload_artifacts = lambda d: "local://" + d
    nc = _build()
    return bass_utils.run_bass_kernel_spmd(
        nc, in_maps, core_ids=list(range(N_CORES)), trace=trace, tmpdir=tmpdir
    )


def _make_in_maps(x, W_expert, W_group):
    x = np.ascontiguousarray(np.asarray(x, dtype=np.float32)).reshape(TOK, D)
    W_all = np.concatenate(
        [np.asarray(W_expert, np.float32), np.asarray(W_group, np.float32)], axis=0
    )
    # pack W^T (D, 18) into the SBUF tile layout (128, N_CHUNKS*18):
    # element (p, k*18+e) = W_all[e, k*128+p]
    wT = _round_f32r(
        np.ascontiguousarray(
            W_all.T.reshape(N_CHUNKS, 128, 18).transpose(1, 0, 2).reshape(128, -1)
        )
    )
    ident = np.eye(128, dtype=np.float32)
    import ml_dtypes
    ident_bf = np.eye(128, dtype=ml_dtypes.bfloat16)
    iota = np.tile(np.arange(16, dtype=np.float32), (128, 16))
    return [
        {"x": x[c * TPC:(c + 1) * TPC], "wT": wT, "ident": ident,
         "ident_bf": ident_bf, "iota": iota}
        for c in range(N_CORES)
    ]


def _gather(results):
    w_parts, i_parts = [], []
    counts = np.zeros(E, dtype=np.float64)
    for c in range(N_CORES):
        r = results[c]
        w_parts.append(
            r["out_w"].reshape(128, N_TILES, 6).transpose(1, 0, 2).reshape(TPC, 6)
        )
        i_parts.append(
            r["out_i"].reshape(128, N_TILES, 6).transpose(1, 0, 2).reshape(TPC, 6)
        )
        counts += r["out_c"].astype(np.float64).reshape(128, N_HALVES, E).sum(axis=(0, 1))
    weights = np.concatenate(w_parts, 0).reshape(B, T, 6).astype(np.float32)
    indices = np.concatenate(i_parts, 0).reshape(B, T, 6).astype(np.int32)
    counts[0] += 2.0 * TOK  # two zero pad slots per token
    actual = counts / counts.sum()
    aux = np.float32(0.01 * np.sum((1.0 / E) * (np.log(1.0 / E) - np.log(actual))))
    return weights, indices, aux


def kernel(x, W_expert, W_group):
    in_maps = _make_in_maps(x, W_expert, W_group)
    res = _run(in_maps)
    return _gather(res.results)


if __name__ == "__main__":
    rng = np.random.default_rng(1)
    x = rng.normal(size=(B, T, D)).astype(np.float32)
    We = (rng.normal(size=(E, D)) * D ** -0.5).astype(np.float32)
    Wg = (rng.normal(size=(2, D)) * D ** -0.5).astype(np.float32)
    w, i, aux = kernel(x, We, Wg)
    print("weights", w.shape, w.dtype, "indices", i.shape, i.dtype, "aux", aux)
